# revision 14
# baseline (speedup 1.0000x reference)
"""ADMM-TV reconstruction (nn_ADMM_TV) as a distributed Bass kernel on 8 trn2 cores.

Strategy: pure data-parallel over batch B=4 (one image per core, cores 0-3;
cores 4-7 run the same program on zeros). The CG inner products couple the
batch through scalar sums, so each CG iteration does one 4-core AllGather of
per-image partial sums (a1=p.Ap, a2=r.Ap, a3=Ap.Ap [, rs0]); rsnew is
reconstructed via rsnew = rsold - 2*alpha*a2 + alpha^2*a3 (algebraically
exact) so a single collective per iteration suffices.

2D centered ortho FFTs are computed as Y = G X G with G = S F S (S=fftshift
permutation, F = ortho DFT matrix; G is symmetric), via two matmul stages per
transform with the DATA as the stationary operand:
  stage1: Z1 = X^T G   (lhsT = X, rhs = G)   => Z1 = (G X)^T
  stage2: Y  = Z1^T G  (lhsT = Z1, rhs = G)
Matmuls run in float32r (12-bit mantissa, 1 cycle/row) with fp32 PSUM
accumulation; validated end-to-end rel err ~2e-4 vs the fp32 reference.
"""
import os
import numpy as np

import concourse.bass as bass
import concourse.bacc as bacc
import concourse.tile as tile
import concourse.mybir as mybir
from concourse.bass_utils import run_bass_kernel_spmd

F32 = mybir.dt.float32
F32R = mybir.dt.float32r
OP = mybir.AluOpType
AX = mybir.AxisListType

N = 320
B = 4
CH = [(0, 128), (128, 128), (256, 64)]  # (row0, nrows) chunks of 320
N_OUTER = 10
N_CG = 10
LAMBDA_TV = 0.01
RHO = 1.0


def _make_g():
    j = np.arange(N)
    F = np.exp(-2j * np.pi * np.outer(j, j) / N) / np.sqrt(N)
    S = np.roll(np.eye(N), N // 2, axis=1)
    G = S @ F @ S
    return G.real.astype(np.float32), G.imag.astype(np.float32)


def build(n_outer=N_OUTER, n_cg=N_CG, cut=99):
    nc = bacc.Bacc("TRN2", target_bir_lowering=False, debug=False, num_devices=8)

    y_ext = nc.dram_tensor("y", [2, N, N], F32, kind="ExternalInput").ap()
    m_ext = nc.dram_tensor("mask", [N, N], F32, kind="ExternalInput").ap()
    gr_ext = nc.dram_tensor("gr", [N, N], F32, kind="ExternalInput").ap()
    gi_ext = nc.dram_tensor("gi", [N, N], F32, kind="ExternalInput").ap()
    out_x = nc.dram_tensor("out_x", [2, N, N], F32, kind="ExternalOutput").ap()
    out_s = nc.dram_tensor("out_s", [1, 8], F32, kind="ExternalOutput").ap()

    RG = [[0, 1, 2, 3], [4, 5, 6, 7]]

    with tile.TileContext(nc) as tc:
        with (
            tc.tile_pool(name="state", bufs=1) as st,
            tc.tile_pool(name="psum", bufs=3, space="PSUM") as psum,
            tc.tile_pool(name="psred", bufs=1, space="PSUM") as psred,
            tc.tile_pool(name="psbc", bufs=1, space="PSUM") as psbc,
            tc.tile_pool(name="dram", bufs=3, space="DRAM") as dram,
        ):
            # ---- persistent state tiles ----
            # complex fields [128, 6, 320]: free idx (comp*3 + chunk, col)
            x = st.tile([128, 6, N], F32, tag="x")
            r = st.tile([128, 6, N], F32, tag="r")
            p = st.tile([128, 6, N], F32, tag="p")
            Ap = st.tile([128, 6, N], F32, tag="Ap")
            b = st.tile([128, 6, N], F32, tag="b")
            b_data = st.tile([128, 6, N], F32, tag="b_data")
            ftmp = st.tile([128, 6, N], F32, tag="ftmp")      # f32 scratch / TTR out
            scr = ftmp
            # 4-channel fields [128, 12, 320]: (ch*3 + chunk, col); ch: dxr,dxi,dyr,dyi
            u_a = st.tile([128, 12, N], F32, tag="u_a")
            u_b = st.tile([128, 12, N], F32, tag="u_b")
            v = st.tile([128, 12, N], F32, tag="v")
            g4 = st.tile([128, 12, N], F32, tag="g4")
            gs = st.tile([128, 12, N], F32, tag="gs")         # g + v
            t1_4 = st.tile([128, 12, N], F32, tag="t1_4")     # soft scratch
            mask = st.tile([128, 3, N], F32, tag="mask")
            tdiv = st.tile([128, 3, N], F32, tag="tdiv")      # divergence result
            dx2 = st.tile([128, 6, N], F32, tag="dx2")        # (dxr, dxi) scratch
            shA = dx2
            shB = st.tile([128, 3, N], F32, tag="shB")        # shift-down scratch (div_y)
            shC = st.tile([128, 3, N], F32, tag="shC")        # zero-last-row copy (div_y)
            zline = st.tile([1, N], F32, tag="zline")         # constant zero row
            # f32r rounded matmul inputs
            inr = st.tile([128, 6, N], F32R, tag="inr")       # rounded stage-1 input
            z1r = st.tile([128, 6, N], F32R, tag="z1r")       # stage-1 out rounded
            mkr = st.tile([128, 6, N], F32R, tag="mkr")       # masked fft rounded
            w1r = z1r  # ifft stage-1 out reuses z1r (dead by then)
            # constants
            c_gr = st.tile([128, 3, N], F32R, tag="c_gr")
            c_gi = st.tile([128, 3, N], F32R, tag="c_gi")
            c_ngi = st.tile([128, 3, N], F32R, tag="c_ngi")
            ones_col = st.tile([128, 1], F32, tag="ones_col")
            ones_row = st.tile([1, 128], F32, tag="ones_row")
            # scalars
            col = st.tile([128, 4], F32, tag="col")           # per-partition partials
            sred = st.tile([8, 1], F32, tag="sred")           # reduced partials (4 scalars)
            agt = st.tile([1, 32], F32, tag="agt")            # AG result
            sums = st.tile([1, 4], F32, tag="sums")           # rank-summed scalars
            scb = st.tile([1, 3], F32, tag="scb")             # alpha, -alpha, beta
            rsold = st.tile([1, 1], F32, tag="rsold")
            sc1 = st.tile([1, 6], F32, tag="sc1")             # scalar scratch
            outsc = st.tile([2, 1], F32, tag="outsc")

            vec = nc.vector
            act = nc.scalar

            # Zero all full-tile-reduced state so the invalid tail rows of the
            # 64-row chunk (partitions 64..127 of chunk 2) stay exactly zero.
            for tl in (x, r, p, Ap, b, b_data, ftmp, u_a, u_b, v, g4, gs,
                       t1_4, tdiv, dx2, shB, shC, mask, col):
                vec.memset(tl[:, :, :] if len(tl.shape) == 3 else tl[:, :], 0.0)
            vec.memset(zline[:, :], 0.0)

            # ================= init: constants =================
            for c, (r0, nr) in enumerate(CH):
                nc.sync.dma_start(out=mask[0:nr, c, :], in_=m_ext[r0:r0 + nr, :])
                nc.sync.dma_start(out=ftmp[0:nr, c, :], in_=gr_ext[r0:r0 + nr, :])
                nc.sync.dma_start(out=ftmp[0:nr, 3 + c, :], in_=gi_ext[r0:r0 + nr, :])
            vec.tensor_copy(c_gr[:, :, :], ftmp[:, 0:3, :])
            vec.tensor_copy(c_gi[:, :, :], ftmp[:, 3:6, :])
            vec.tensor_scalar_mul(c_ngi[:, :, :], ftmp[:, 3:6, :], -1.0)
            vec.memset(ones_col[:, :], 1.0)
            vec.memset(ones_row[:, :], 1.0)
            vec.memset(outsc[:, :], 0.0)
            vec.memset(sred[:, :], 0.0)

            # ================= helpers =================
            FFT_COMBO = ((0, c_gr), (1, c_ngi), (0, c_gi), (1, c_gr))
            IFFT_COMBO = ((0, c_gr), (1, c_gi), (1, c_gr), (0, c_ngi))

            def cstage(src_r, combo, writer):
                """One complex matmul stage: out = src^T (G-combo); writer(s, ms, ps_re, ps_im)."""
                (a_c, a_g), (b_c, b_g), (c_c, c_g), (d_c, d_g) = combo
                for s, (m0, ms) in enumerate(CH):
                    ps_re = psum.tile([128, N], F32, tag="ps_re")
                    ps_im = psum.tile([128, N], F32, tag="ps_im")
                    for c, (k0, ks) in enumerate(CH):
                        nc.tensor.matmul(ps_re[0:ms, :], src_r[0:ks, 3 * a_c + c, m0:m0 + ms],
                                         a_g[0:ks, c, :], start=(c == 0), stop=False)
                        nc.tensor.matmul(ps_re[0:ms, :], src_r[0:ks, 3 * b_c + c, m0:m0 + ms],
                                         b_g[0:ks, c, :], start=False, stop=(c == 2))
                        nc.tensor.matmul(ps_im[0:ms, :], src_r[0:ks, 3 * c_c + c, m0:m0 + ms],
                                         c_g[0:ks, c, :], start=(c == 0), stop=False)
                        nc.tensor.matmul(ps_im[0:ms, :], src_r[0:ks, 3 * d_c + c, m0:m0 + ms],
                                         d_g[0:ks, c, :], start=False, stop=(c == 2))
                    writer(s, ms, ps_re, ps_im)

            def w_round_to(dst):
                def w(s, ms, ps_re, ps_im):
                    act.copy(dst[0:ms, s, :], ps_re[0:ms, :])
                    act.copy(dst[0:ms, 3 + s, :], ps_im[0:ms, :])
                return w

            def w_mask_to(dst):
                def w(s, ms, ps_re, ps_im):
                    vec.tensor_mul(dst[0:ms, s, :], ps_re[0:ms, :], mask[0:ms, s, :])
                    vec.tensor_mul(dst[0:ms, 3 + s, :], ps_im[0:ms, :], mask[0:ms, s, :])
                return w

            def w_copy_to(dst):
                def w(s, ms, ps_re, ps_im):
                    act.copy(dst[0:ms, s, :], ps_re[0:ms, :])
                    act.copy(dst[0:ms, 3 + s, :], ps_im[0:ms, :])
                return w

            def w_sub_t(dst):
                # dst = ifft_result - tdiv (broadcast over both components)
                def w(s, ms, ps_re, ps_im):
                    vec.tensor_sub(dst[0:ms, s, :], ps_re[0:ms, :], tdiv[0:ms, s, :])
                    vec.tensor_sub(dst[0:ms, 3 + s, :], ps_im[0:ms, :], tdiv[0:ms, s, :])
                return w

            def emit_dx2(z):
                """dx2 = forward x-diff of both components of z (the A-operator gradient part)."""
                vec.tensor_sub(dx2[:, :, 0:N - 1], z[:, 0:6, 1:N], z[:, 0:6, 0:N - 1])
                vec.memset(dx2[:, :, N - 1:N], 0.0)

            def emit_shift_down(dst, src, cb_dst, cb_src):
                """dst[row p] = src[row p-1] (p=1..319), dst[row 0] = 0; one 320-row channel."""
                vec.memset(dst[0:1, cb_dst, :], 0.0)
                nc.sync.dma_start(out=dst[1:128, cb_dst, :], in_=src[0:127, cb_src, :])
                nc.sync.dma_start(out=dst[0:1, cb_dst + 1, :], in_=src[127:128, cb_src, :])
                nc.sync.dma_start(out=dst[1:128, cb_dst + 1, :], in_=src[0:127, cb_src + 1, :])
                nc.sync.dma_start(out=dst[0:1, cb_dst + 2, :], in_=src[127:128, cb_src + 1, :])
                nc.sync.dma_start(out=dst[1:64, cb_dst + 2, :], in_=src[0:63, cb_src + 2, :])

            def emit_copy_zlast(dst, src, cb_dst, cb_src):
                """dst[row p] = src[row p] (p<=318), dst[row 319] = 0; one channel."""
                nc.sync.dma_start(out=dst[0:128, cb_dst, :], in_=src[0:128, cb_src, :])
                nc.sync.dma_start(out=dst[0:128, cb_dst + 1, :], in_=src[0:128, cb_src + 1, :])
                nc.sync.dma_start(out=dst[0:63, cb_dst + 2, :], in_=src[0:63, cb_src + 2, :])
                nc.sync.dma_start(out=dst[63:64, cb_dst + 2, :], in_=zline[0:1, :])

            def emit_div(px_t, px0, py_t, py0, out_t):
                """out = div_x(px) + div_y(py); px/py: (tile, chunk-base), 3 chunks each."""
                # div_x into out
                vec.tensor_copy(out_t[:, :, 0:1], px_t[:, px0:px0 + 3, 0:1])
                vec.tensor_sub(out_t[:, :, 1:N - 1], px_t[:, px0:px0 + 3, 1:N - 1],
                               px_t[:, px0:px0 + 3, 0:N - 2])
                vec.tensor_scalar_mul(out_t[:, :, N - 1:N],
                                      px_t[:, px0:px0 + 3, N - 2:N - 1], -1.0)
                # div_y = pyz - pysh, added into out
                emit_copy_zlast(shC, py_t, 0, py0)
                emit_shift_down(shB, py_t, 0, py0)
                vec.tensor_add(out_t[:, :, :], out_t[:, :, :], shC[:, :, :])
                vec.tensor_sub(out_t[:, :, :], out_t[:, :, :], shB[:, :, :])

            def emit_A(z_f32, dst):
                """dst = ifft2c(mask*fft2c(z)) - div_x(dx_re(z)) - div_y(dx_im(z))  (RHO=1)."""
                act.copy(inr[:, :, :], z_f32[:, 0:6, :])
                emit_dx2(z_f32)
                emit_div(dx2, 0, dx2, 3, tdiv)
                cstage(inr, FFT_COMBO, w_round_to(z1r))
                cstage(z1r, FFT_COMBO, w_mask_to(mkr))
                cstage(mkr, IFFT_COMBO, w_round_to(w1r))
                cstage(w1r, IFFT_COMBO, w_sub_t(dst))

            def emit_shift_up(dst, src, cb_dst, cb_src):
                """dst[row p] = src[row p+1] (p<=318), dst[row 319] = src[row 319]."""
                nc.sync.dma_start(out=dst[0:127, cb_dst, :], in_=src[1:128, cb_src, :])
                nc.sync.dma_start(out=dst[127:128, cb_dst, :], in_=src[0:1, cb_src + 1, :])
                nc.sync.dma_start(out=dst[0:127, cb_dst + 1, :], in_=src[1:128, cb_src + 1, :])
                nc.sync.dma_start(out=dst[127:128, cb_dst + 1, :], in_=src[0:1, cb_src + 2, :])
                nc.sync.dma_start(out=dst[0:63, cb_dst + 2, :], in_=src[1:64, cb_src + 2, :])
                nc.sync.dma_start(out=dst[63:64, cb_dst + 2, :], in_=src[63:64, cb_src + 2, :])

            def emit_grad4(z, dst):
                """dst[ch] = (dx_re, dx_im, dy_re, dy_im) of complex field z."""
                vec.tensor_sub(dst[:, 0:6, 0:N - 1], z[:, 0:6, 1:N], z[:, 0:6, 0:N - 1])
                vec.memset(dst[:, 0:6, N - 1:N], 0.0)
                emit_shift_up(shA, z, 0, 0)
                emit_shift_up(shA, z, 3, 3)
                vec.tensor_sub(dst[:, 6:12, :], shA[:, 0:6, :], z[:, 0:6, :])

            def cg_reduce_round(n_scal, first):
                """Reduce col[:, 0:n_scal] across partitions and ranks into sums[0:1, 0:n_scal]."""
                red = psred.tile([4, 1], F32, tag="red")
                nc.tensor.matmul(red[0:n_scal, 0:1], col[:, 0:n_scal], ones_col[:, :],
                                 start=True, stop=True)
                act.copy(sred[0:n_scal, 0:1], red[0:n_scal, 0:1])
                cc_in = dram.tile([1, 8], F32, tag="cc_in")
                cc_out = dram.tile([4, 8], F32, tag="cc_out")
                nc.sync.dma_start(out=cc_in[0:1, 0:8], in_=sred[0:8, 0:1])
                nc.gpsimd.collective_compute(
                    "AllGather", OP.bypass, replica_groups=RG,
                    ins=[cc_in[:, :].opt()], outs=[cc_out[:, :].opt()])
                nc.sync.dma_start(out=agt[:, :], in_=cc_out[:, :])
                rview = agt[0:1, :].rearrange("p (r e) -> p e r", r=4)
                vec.tensor_reduce(sums[0:1, 0:n_scal], rview[0:1, 0:n_scal, :], axis=AX.X, op=OP.add)

            # ================= init: data =================
            for c, (r0, nr) in enumerate(CH):
                nc.sync.dma_start(out=ftmp[0:nr, c, :], in_=y_ext[0, r0:r0 + nr, :])
                nc.sync.dma_start(out=ftmp[0:nr, 3 + c, :], in_=y_ext[1, r0:r0 + nr, :])
            # x0 = ifft2c(y)
            act.copy(inr[:, :, :], ftmp[:, 0:6, :])
            cstage(inr, IFFT_COMBO, w_round_to(z1r))
            cstage(z1r, IFFT_COMBO, w_copy_to(x))
            # b_data = ifft2c(mask*y)
            vec.tensor_mul(dx2[:, 0:3, :], ftmp[:, 0:3, :], mask[:, :, :])
            vec.tensor_mul(dx2[:, 3:6, :], ftmp[:, 3:6, :], mask[:, :, :])
            act.copy(inr[:, :, :], dx2[:, 0:6, :])
            cstage(inr, IFFT_COMBO, w_round_to(z1r))
            cstage(z1r, IFFT_COMBO, w_copy_to(b_data))
            # u0 = gradient(x0); v0 = 0
            emit_grad4(x, u_a)
            vec.memset(v[:, :, :], 0.0)

            u_cur, u_nxt = u_a, u_b

            # ================= outer ADMM loop =================
            for it_out in range(n_outer if cut >= 2 else 0):
                # b = b_data + div(u - v) on channels 0,1
                vec.tensor_sub(dx2[:, 0:6, :], u_cur[:, 0:6, :], v[:, 0:6, :])
                emit_div(dx2, 0, dx2, 3, tdiv)
                vec.tensor_add(b[:, 0:3, :], b_data[:, 0:3, :], tdiv[:, :, :])
                vec.tensor_add(b[:, 3:6, :], b_data[:, 3:6, :], tdiv[:, :, :])

                # r0 = b - A(x); p0 = r0; rs0 partial
                if cut >= 3:
                    emit_A(x, Ap)
                if cut >= 4:
                    vec.tensor_sub(r[:, :, :], b[:, :, :], Ap[:, :, :])
                    vec.tensor_copy(p[:, :, :], r[:, :, :])
                    vec.scalar_tensor_tensor(scr[:, :, :], r[:, :, :], 1.0, r[:, :, :],
                                             OP.mult, OP.mult, accum_out=col[:, 3:4])

                for it in range(n_cg if cut >= 5 else 0):
                    emit_A(p, Ap)
                    vec.scalar_tensor_tensor(scr[:, :, :], p[:, :, :], 1.0, Ap[:, :, :],
                                             OP.mult, OP.mult, accum_out=col[:, 0:1])
                    vec.scalar_tensor_tensor(scr[:, :, :], r[:, :, :], 1.0, Ap[:, :, :],
                                             OP.mult, OP.mult, accum_out=col[:, 1:2])
                    vec.scalar_tensor_tensor(scr[:, :, :], Ap[:, :, :], 1.0, Ap[:, :, :],
                                             OP.mult, OP.mult, accum_out=col[:, 2:3])
                    if cut < 6:
                        continue
                    cg_reduce_round(4 if it == 0 else 3, it == 0)
                    if it == 0:
                        vec.tensor_copy(rsold[0:1, 0:1], sums[0:1, 3:4])
                    # alpha = rsold / (a1 + 1e-12)
                    vec.tensor_scalar_add(sc1[0:1, 0:1], sums[0:1, 0:1], 1e-12)
                    vec.reciprocal(sc1[0:1, 1:2], sc1[0:1, 0:1])
                    vec.tensor_mul(scb[0:1, 0:1], rsold[0:1, 0:1], sc1[0:1, 1:2])
                    vec.tensor_scalar_mul(scb[0:1, 1:2], scb[0:1, 0:1], -1.0)
                    # rsnew = rsold - 2*alpha*a2 + alpha^2*a3
                    vec.tensor_mul(sc1[0:1, 2:3], scb[0:1, 0:1], sums[0:1, 1:2])      # alpha*a2
                    vec.tensor_mul(sc1[0:1, 3:4], scb[0:1, 0:1], scb[0:1, 0:1])       # alpha^2
                    vec.tensor_mul(sc1[0:1, 4:5], sc1[0:1, 3:4], sums[0:1, 2:3])      # alpha^2*a3
                    vec.scalar_tensor_tensor(sc1[0:1, 5:6], sc1[0:1, 2:3], -2.0,
                                             rsold[0:1, 0:1], OP.mult, OP.add)        # rsold-2*alpha*a2
                    vec.tensor_add(sc1[0:1, 5:6], sc1[0:1, 5:6], sc1[0:1, 4:5])       # rsnew
                    # beta = rsnew / (rsold + 1e-30)
                    vec.tensor_scalar_add(sc1[0:1, 0:1], rsold[0:1, 0:1], 1e-30)
                    vec.reciprocal(sc1[0:1, 1:2], sc1[0:1, 0:1])
                    vec.tensor_mul(scb[0:1, 2:3], sc1[0:1, 5:6], sc1[0:1, 1:2])
                    vec.tensor_copy(rsold[0:1, 0:1], sc1[0:1, 5:6])
                    if cut < 7:
                        continue
                    # broadcast (alpha, -alpha, beta) to 128 partitions
                    bc = psbc.tile([128, 3], F32, tag="bc")
                    nc.tensor.matmul(bc[:, 0:3], ones_row[0:1, :], scb[0:1, 0:3],
                                     start=True, stop=True)
                    # x += alpha p ; r -= alpha Ap ; p = beta p + r
                    vec.scalar_tensor_tensor(x[:, :, :], p[:, :, :], bc[:, 0:1], x[:, :, :],
                                             OP.mult, OP.add)
                    vec.scalar_tensor_tensor(r[:, :, :], Ap[:, :, :], bc[:, 1:2], r[:, :, :],
                                             OP.mult, OP.add)
                    vec.scalar_tensor_tensor(p[:, :, :], p[:, :, :], bc[:, 2:3], r[:, :, :],
                                             OP.mult, OP.add)

                # g = gradient(x); un = soft(g+v); v += g - un
                if cut < 8:
                    continue
                emit_grad4(x, g4)
                vec.tensor_add(gs[:, :, :], g4[:, :, :], v[:, :, :])
                vec.tensor_scalar(t1_4[:, :, :], gs[:, :, :], LAMBDA_TV / RHO, 0.0,
                                  OP.subtract, OP.max)
                vec.tensor_scalar(u_nxt[:, :, :], gs[:, :, :], LAMBDA_TV / RHO, 0.0,
                                  OP.add, OP.min)
                vec.tensor_add(u_nxt[:, :, :], u_nxt[:, :, :], t1_4[:, :, :])
                vec.tensor_sub(v[:, :, :], gs[:, :, :], u_nxt[:, :, :])

                if it_out == n_outer - 1:
                    # pres^2 partial = |g - un|^2 ; dres^2 partial = |div(un - u_old)|^2
                    vec.tensor_sub(t1_4[:, :, :], g4[:, :, :], u_nxt[:, :, :])
                    vec.scalar_tensor_tensor(gs[:, :, :], t1_4[:, :, :], 1.0, t1_4[:, :, :],
                                             OP.mult, OP.mult, accum_out=col[:, 0:1])
                    vec.tensor_sub(dx2[:, 0:6, :], u_nxt[:, 0:6, :], u_cur[:, 0:6, :])
                    emit_div(dx2, 0, dx2, 3, tdiv)
                    vec.scalar_tensor_tensor(dx2[:, 0:3, :], tdiv[:, :, :], 1.0, tdiv[:, :, :],
                                             OP.mult, OP.mult, accum_out=col[:, 1:2])
                    red2 = psred.tile([4, 1], F32, tag="red")
                    nc.tensor.matmul(red2[0:2, 0:1], col[:, 0:2], ones_col[:, :],
                                     start=True, stop=True)
                    act.copy(outsc[0:2, 0:1], red2[0:2, 0:1])

                u_cur, u_nxt = u_nxt, u_cur

            # ================= outputs =================
            nc.sync.dma_start(out=out_s[0:1, 0:2], in_=outsc[0:2, 0:1])
            for c, (r0, nr) in enumerate(CH):
                nc.sync.dma_start(out=out_x[0, r0:r0 + nr, :], in_=x[0:nr, c, :])
                nc.sync.dma_start(out=out_x[1, r0:r0 + nr, :], in_=x[0:nr, 3 + c, :])

    nc.compile()
    return nc


_NC_CACHE = {}


def _get_nc(n_outer=N_OUTER, n_cg=N_CG, cut=99):
    key = (n_outer, n_cg, cut)
    if key not in _NC_CACHE:
        _NC_CACHE[key] = build(n_outer, n_cg, cut)
    return _NC_CACHE[key]


def run(y, mask, n_outer=N_OUTER, n_cg=N_CG, trace=False, tmpdir=None, cut=99):
    """y (4,2,320,320) f32, mask (4,1,320,320) f32 -> (x, count, pres, dres), results obj."""
    gr, gi = _make_g()
    nc = _get_nc(n_outer, n_cg, cut)
    zero_y = np.zeros((2, N, N), np.float32)
    zero_m = np.zeros((N, N), np.float32)
    in_maps = []
    for core in range(8):
        if core < B:
            in_maps.append({"y": np.ascontiguousarray(y[core]),
                            "mask": np.ascontiguousarray(mask[core, 0]),
                            "gr": gr, "gi": gi})
        else:
            in_maps.append({"y": zero_y, "mask": zero_m, "gr": gr, "gi": gi})
    res = run_bass_kernel_spmd(nc, in_maps, core_ids=list(range(8)),
                               trace=trace, tmpdir=tmpdir)
    xs = np.stack([res.results[i]["out_x"] for i in range(B)], axis=0)
    pres2 = sum(float(res.results[i]["out_s"][0, 0]) for i in range(B))
    dres2 = sum(float(res.results[i]["out_s"][0, 1]) for i in range(B))
    count = np.array(n_outer, np.int32)
    pres = np.float32(np.sqrt(pres2))
    dres = np.float32(RHO * np.sqrt(dres2))
    return (xs, count, pres, dres), res


def kernel(y, mask):
    (xs, count, pres, dres), _ = run(y, mask)
    return xs, count, pres, dres


if __name__ == "__main__":
    rng = np.random.default_rng(0)
    y = rng.standard_normal((B, 2, N, N)).astype(np.float32)
    mask = (rng.random((B, 1, N, N)) < 0.3).astype(np.float32)
    out = kernel(y, mask)
    print([np.asarray(o).shape for o in out])
